# revision 1
# baseline (speedup 1.0000x reference)
"""Cox partial-likelihood NegativeLogLikelihood loss on 8 Trainium2 cores.

reference:
    mask[i, j] = (y[j] <= y[i])                       # (N, N)
    num[j] = sum_i exp(r_i) * mask[i, j]
    den[j] = sum_i mask[i, j]
    loss = -sum_j e_j * (r_j - log(num_j / den_j)) / sum_j e_j + 0.01 * ||W||_F

Strategy: shard columns j across the 8 cores (each core owns 2048 columns and
reads the full y / risk_pred, which are tiny).  Per core, the N x 2048 mask is
generated on-chip in [128, 2048] tiles (DVE tensor_scalar is_le, with a share
of tiles on ACT via the double-Sign identity sign(sign(y_i - y_j) + 1)) and
immediately contracted on the TensorEngine against lhsT = [exp_hi, exp_lo, 1]
(bf16, exp = hi + lo Dekker split for f32-grade accuracy) accumulating into
PSUM.  Each core reduces its own columns to a single partial scalar
out_c = -t_c / e_sum + 0.01 * ||W||_F / 8  (e_sum, ||W|| computed redundantly
from the replicated inputs), so the host-side unshard is a pure 8-way sum.
"""
import math

import numpy as np
import orjson

import concourse.bass as bass
import concourse.tile as tile
import concourse.mybir as mybir
from concourse.bass_utils import run_bass_kernel_spmd

F32 = mybir.dt.float32
BF16 = mybir.dt.bfloat16
I32 = mybir.dt.int32

N = 16384
NCORES = 8
JSHARD = N // NCORES            # 2048 columns per core
NT = N // 128                   # 128 i-tiles of 128 rows
NJJ = JSHARD // 512             # 4 matmul column groups per core
NCHUNKED = 12                   # first tiles produced per-512-chunk
N_ACT_FULL = 43                 # full-width ScalarE tiles (rest on DVE)
ACT_TILES = sorted({t for t in range(NCHUNKED) if t % 2 == 1} |
                   {NCHUNKED + round((k + 0.5) * (NT - NCHUNKED) / N_ACT_FULL)
                    for k in range(N_ACT_FULL)})
ACT_SET = set(ACT_TILES)
# ScalarE tiles feed s = sign(y_i - y_j) in {-1,0,1} to the matmul with a
# HALVED lhsT; since s = 2*mask - 1 - [y_i == y_j], the column sums are fixed
# afterwards by adding V_ACT/2 (sum of exp over ACT-tile rows), 0.5*C_ACT to
# den, and the diagonal term exp_j/2 (resp. 0.5) for columns whose own row
# lives in an ACT tile.  Off-diagonal exact ties inside ACT tiles (expected
# ~8 pairs in 16K uniform floats) contribute O(1e-6) relative error.
C_ACT_HALF = float(len(ACT_TILES) * 128) / 2.0

# ---------------------------------------------------------------------------
# Workaround for the installed walrus accepting at most ONE sync-wait command
# per TPB instruction: split multi-wait instructions into preceding
# single-wait EventSemaphore instructions on the same engine.
# ---------------------------------------------------------------------------

def _fix_bir_multiwait(bir_json: bytes) -> bytes:
    d = orjson.loads(bir_json)
    counter = 0
    for fn in d.get("functions", []):
        stack = list(fn.get("blocks", []))
        while stack:
            block = stack.pop()
            stack.extend(block.get("blocks", []))
            new_insts = []
            for inst in block.get("instructions", []):
                sync = inst.get("sync_info") or {}
                waits = sync.get("on_wait") or []
                if len(waits) > 1:
                    for w in waits[:-1]:
                        counter += 1
                        new_insts.append({
                            "debug": inst.get("debug", 0),
                            "engine": inst.get("engine"),
                            "ins": [],
                            "name": f"esw_fix_{counter}",
                            "opcode": "EventSemaphore",
                            "outs": [],
                            "sync_info": {"on_update": [], "on_wait": [w]},
                        })
                    sync["on_wait"] = [waits[-1]]
                new_insts.append(inst)
            block["instructions"] = new_insts
    return orjson.dumps(d)


_patched = False


def _install_bir_fix():
    global _patched
    if _patched:
        return
    _patched = True
    import concourse.bass_utils as bu
    import concourse.bass2jax as b2j

    orig = bu.compile_bir_kernel

    def patched(bir_json, tmpdir, neff_name="file.neff"):
        if isinstance(bir_json, str):
            bir_json = bir_json.encode()
        return orig(_fix_bir_multiwait(bir_json), tmpdir, neff_name)

    bu.compile_bir_kernel = patched
    b2j.compile_bir_kernel = patched


# ---------------------------------------------------------------------------
# Kernel build
# ---------------------------------------------------------------------------

def build_kernel() -> bass.Bass:
    nc = bass.Bass()
    Sign = mybir.ActivationFunctionType.Sign

    # crit: [y_col | r_col] (full y and risk in column-major [p, t] layout)
    crit = nc.dram_tensor("crit", [128, 2 * NT], F32, kind="ExternalInput")
    y_row = nc.dram_tensor("y_row", [1, JSHARD], F32, kind="ExternalInput")
    # misc: [r_pf | e_pf | e_all | w_t | scale_b | indhalf_b | ind_pf_half]
    MISC_W = NJJ * 4 + NJJ * 4 + NT + 1024 + NT + NT + NJJ * 4
    misc = nc.dram_tensor("misc", [128, MISC_W], F32, kind="ExternalInput")
    out = nc.dram_tensor("out", [1, 1], F32, kind="ExternalOutput")

    with tile.TileContext(nc) as tc:
        with (
            tc.tile_pool(name="const", bufs=1) as const,
            tc.tile_pool(name="masks", bufs=12) as masks,
            tc.tile_pool(name="psacc", bufs=1, space="PSUM") as psacc,
            tc.tile_pool(name="pswarm", bufs=1, space="PSUM") as pswarm,
            tc.tile_pool(name="pssum", bufs=1, space="PSUM") as pssum,
        ):
            # ---- critical-path loads: crit pack first, then y broadcast
            crit_sb = const.tile([128, 2 * NT], F32)
            nc.sync.dma_start(out=crit_sb, in_=crit[:, :])
            MISC_W = NJJ * 4 + NJJ * 4 + NT + 1024 + NT + NT + NJJ * 4
            ycol_sb = crit_sb[:, 0:NT]
            rcol_sb = crit_sb[:, NT:2 * NT]
            yb = const.tile([128, JSHARD], F32)
            for q in range(NJJ):
                eng = nc.gpsimd if q % 2 == 0 else nc.sync
                eng.dma_start(
                    out=yb[:, 512 * q:512 * (q + 1)],
                    in_=y_row.ap()[:, 512 * q:512 * (q + 1)].to_broadcast([128, 512]),
                )

            # ---- misc pack slices (DMA is issued below, after the loop)
            misc_sb = const.tile([128, MISC_W], F32)
            nc.sync.dma_start(out=misc_sb, in_=misc[:, :])
            o = 0
            rpf_sb = misc_sb[:, o:o + NJJ * 4]; o += NJJ * 4
            epf_f = misc_sb[:, o:o + NJJ * 4]; o += NJJ * 4
            e_f = misc_sb[:, o:o + NT]; o += NT
            w_sb = misc_sb[:, o:o + 1024]; o += 1024
            scale_b = misc_sb[:, o:o + NT]; o += NT      # 0.5 on ACT cols else 1
            indh_b = misc_sb[:, o:o + NT]; o += NT       # 0.5 on ACT cols else 0
            indpf_sb = misc_sb[:, o:o + NJJ * 4]; o += NJJ * 4

            # ---- lhsT = scale * [exp_hi | exp_lo | ones] per i-tile, bf16
            exp_sb = const.tile([128, NT], F32)
            nc.scalar.activation(exp_sb, rcol_sb, mybir.ActivationFunctionType.Exp)
            lhsT = const.tile([128, 3, NT], BF16)
            nc.vector.tensor_copy(lhsT[:, 0, :], exp_sb)          # hi = bf16(exp)
            hi32 = const.tile([128, NT], F32)
            nc.vector.tensor_copy(hi32, lhsT[:, 0, :])            # back to f32
            lo32 = const.tile([128, NT], F32)
            nc.vector.tensor_sub(lo32, exp_sb, hi32)              # f32 residual
            nc.vector.tensor_mul(lhsT[:, 0, :], hi32, scale_b)    # exact in bf16
            nc.vector.tensor_mul(lhsT[:, 1, :], lo32, scale_b)
            nc.vector.tensor_copy(lhsT[:, 2, :], scale_b)

            # ---- V_ACT/2 = sum(exp * indh) -> scalar -> broadcast to [128,1]
            vh = const.tile([128, NT], F32)
            nc.vector.tensor_mul(vh, exp_sb, indh_b)
            vred = const.tile([128, 1], F32)
            nc.vector.tensor_reduce(
                out=vred, in_=vh, axis=mybir.AxisListType.X, op=mybir.AluOpType.add)
            ones_col = const.tile([128, 1], F32)
            nc.vector.memset(ones_col, 1.0)
            va_ps = pssum.tile([1, 1], F32, name="va_ps")
            nc.tensor.matmul(va_ps[:, :], vred, ones_col, start=True, stop=True)
            va_row = const.tile([1, 1], F32)
            nc.vector.tensor_copy(va_row, va_ps[:, :])
            va_b = const.tile([128, 1], F32)
            va_dram = nc.dram_tensor("va_scratch", [1, 1], F32, kind="Internal")
            nc.gpsimd.dma_start(out=va_dram[:, :], in_=va_row)
            nc.gpsimd.dma_start(out=va_b, in_=va_dram.ap().to_broadcast([128, 1]))

            # ---- PE HAM warm-up: dummy matmuls so the real stream starts hot
            warm_ps = pswarm.tile([1, 256], F32)
            for k in range(4):
                nc.tensor.matmul(
                    warm_ps[:, :], ones_col, crit_sb[:, 0:256],
                    start=True, stop=True, skip_group_check=True,
                )

            # ---- main loop: mask tiles + matmul accumulation
            acc = psacc.tile([3, NJJ * 512], F32)
            for t in range(NT):
                m = masks.tile([128, JSHARD], BF16)
                if t < NCHUNKED:
                    for jj in range(NJJ):
                        if t in ACT_SET:
                            nc.scalar.activation(
                                m[:, 512 * jj:512 * (jj + 1)],
                                yb[:, 512 * jj:512 * (jj + 1)],
                                Sign, bias=ycol_sb[:, t:t + 1], scale=-1.0,
                            )
                        else:
                            nc.vector.tensor_scalar(
                                out=m[:, 512 * jj:512 * (jj + 1)],
                                in0=yb[:, 512 * jj:512 * (jj + 1)],
                                scalar1=ycol_sb[:, t:t + 1], scalar2=None,
                                op0=mybir.AluOpType.is_le,
                            )
                elif t in ACT_SET:
                    nc.scalar.activation(
                        m, yb, Sign, bias=ycol_sb[:, t:t + 1], scale=-1.0,
                    )
                else:
                    nc.vector.tensor_scalar(
                        out=m, in0=yb, scalar1=ycol_sb[:, t:t + 1], scalar2=None,
                        op0=mybir.AluOpType.is_le,
                    )
                for jj in range(NJJ):
                    nc.tensor.matmul(
                        acc[:, 512 * jj:512 * (jj + 1)], lhsT[:, :, t],
                        m[:, 512 * jj:512 * (jj + 1)],
                        start=(t == 0), stop=(t == NT - 1),
                    )

            # ---- non-critical reductions (scheduled during main loop)
            vec3 = const.tile([128, 3], F32)
            nc.vector.tensor_reduce(
                out=vec3[:, 0:1], in_=e_f, axis=mybir.AxisListType.X,
                op=mybir.AluOpType.add,
            )
            w2d = const.tile([128, 1024], F32)
            nc.scalar.activation(
                w2d, w_sb, mybir.ActivationFunctionType.Square,
                accum_out=vec3[:, 1:2],
            )

            # ---- epilogue: one psum->sbuf copy, then 3 whole-row scatters
            # pf mapping: x_pf[p, c] = x_shard[16*p + c]
            hi_pf = const.tile([128, NJJ * 4], F32)
            lo_pf = const.tile([128, NJJ * 4], F32)
            den_pf = const.tile([128, NJJ * 4], F32)
            nd_all = const.tile([3, NJJ * 512], F32)
            nc.scalar.copy(nd_all[:, 0:1024], acc[:, 0:1024])
            nc.vector.tensor_copy(nd_all[:, 1024:2048], acc[:, 1024:2048])
            nc.gpsimd.dma_start(out=hi_pf, in_=nd_all[0:1, :])
            nc.sync.dma_start(out=lo_pf, in_=nd_all[1:2, :])
            nc.gpsimd.dma_start(out=den_pf, in_=nd_all[2:3, :])

            # ---- wide final math on [128, 16] with s-encoding corrections
            exp_pf = const.tile([128, NJJ * 4], F32)
            nc.scalar.activation(exp_pf, rpf_sb, mybir.ActivationFunctionType.Exp)
            dterm = const.tile([128, NJJ * 4], F32)
            nc.vector.tensor_mul(dterm, exp_pf, indpf_sb)         # exp_j/2 * ind
            n1 = const.tile([128, NJJ * 4], F32)
            nc.vector.tensor_add(n1, hi_pf, lo_pf)
            n2 = const.tile([128, NJJ * 4], F32)
            nc.vector.tensor_scalar(
                out=n2, in0=n1, scalar1=va_b[:, 0:1], scalar2=None,
                op0=mybir.AluOpType.add)                          # + V_ACT/2
            num_pf = const.tile([128, NJJ * 4], F32)
            nc.vector.tensor_add(num_pf, n2, dterm)
            d1 = const.tile([128, NJJ * 4], F32)
            nc.vector.tensor_scalar(
                out=d1, in0=den_pf, scalar1=C_ACT_HALF, scalar2=None,
                op0=mybir.AluOpType.add)                          # + C_ACT/2
            den2_pf = const.tile([128, NJJ * 4], F32)
            nc.vector.tensor_add(den2_pf, d1, indpf_sb)           # + 0.5*ind
            lnn = const.tile([128, NJJ * 4], F32)
            nc.scalar.activation(lnn, num_pf, mybir.ActivationFunctionType.Ln)
            lnd = const.tile([128, NJJ * 4], F32)
            nc.scalar.activation(lnd, den2_pf, mybir.ActivationFunctionType.Ln)
            s1 = const.tile([128, NJJ * 4], F32)
            nc.vector.tensor_sub(s1, rpf_sb, lnn)
            s2 = const.tile([128, NJJ * 4], F32)
            nc.vector.tensor_add(s2, s1, lnd)
            s3 = const.tile([128, NJJ * 4], F32)
            nc.vector.tensor_mul(s3, s2, epf_f)
            nc.vector.tensor_reduce(
                out=vec3[:, 2:3], in_=s3, axis=mybir.AxisListType.X,
                op=mybir.AluOpType.add,
            )

            # ---- cross-partition fold: [e_sum, w_ssq, t_sum] into one row
            sums = pssum.tile([1, 3], F32)
            nc.tensor.matmul(sums[:, :], ones_col, vec3[:, :], start=True, stop=True)

            # ---- assemble out_c = -t_sum / e_sum + (0.01/8) * sqrt(w_ssq)
            inv_e = const.tile([1, 1], F32)
            nc.vector.reciprocal(inv_e, sums[0:1, 0:1])
            lnw = const.tile([1, 1], F32)
            nc.scalar.activation(lnw, sums[0:1, 1:2], mybir.ActivationFunctionType.Ln)
            f1 = const.tile([1, 1], F32)
            # 0.00125 * sqrt(w_ssq) = exp(0.5 * ln(w_ssq) + ln(0.00125))
            lbias = const.tile([1, 1], F32)
            nc.vector.memset(lbias, math.log(0.01 / NCORES))
            nc.scalar.activation(
                f1, lnw, mybir.ActivationFunctionType.Exp,
                scale=0.5, bias=lbias,
            )
            tsc = const.tile([1, 1], F32)
            nc.vector.tensor_mul(tsc, sums[0:1, 2:3], inv_e)
            res = const.tile([1, 1], F32)
            nc.vector.tensor_sub(res, f1, tsc)
            nc.gpsimd.dma_start(out=out[:, :], in_=res)

    return nc


_nc_cache = None


def _get_nc():
    global _nc_cache
    if _nc_cache is None:
        _install_bir_fix()
        _nc_cache = build_kernel()
    return _nc_cache


def make_in_maps(risk_pred, y, e, W):
    """Host-side sharding: slice/reshape the full inputs for each core."""
    yf = y.reshape(NT, 128).T                                # y_col[p,t]=y[t*128+p]
    rf = risk_pred.reshape(NT, 128).T
    ef = e.astype(np.float32).reshape(NT, 128).T             # e is 0/1: exact in f32
    crit = np.ascontiguousarray(np.concatenate([yf, rf], axis=1))
    w_flat = W.reshape(128, 1024)
    act_mask = np.zeros(NT, np.float32)
    act_mask[list(ACT_SET)] = 1.0
    scale_b = np.tile(1.0 - 0.5 * act_mask, (128, 1)).astype(np.float32)
    indh_b = np.tile(0.5 * act_mask, (128, 1)).astype(np.float32)
    # ind_pf_half[c][p, 4*jj+t] = 0.5 if ((c*JSHARD + 512*jj + 4*p + t)//128) in ACT_SET
    ind_pf_half = []
    for c in range(NCORES):
        j_idx = (c * JSHARD + np.arange(JSHARD)) // 128
        ind = 0.5 * np.isin(j_idx, list(ACT_SET)).astype(np.float32)
        ind_pf_half.append(np.ascontiguousarray(ind.reshape(128, NJJ * 4)))

    in_maps = []
    for c in range(NCORES):
        j0 = c * JSHARD
        ysh = y.reshape(-1)[j0:j0 + JSHARD]
        rsh = risk_pred.reshape(-1)[j0:j0 + JSHARD]
        esh = e.astype(np.float32).reshape(-1)[j0:j0 + JSHARD]
        # pf layout: x_pf[p, c] = x_shard[16*p + c]
        r_pf = rsh.reshape(128, NJJ * 4)
        e_pf = esh.reshape(128, NJJ * 4)
        misc = np.ascontiguousarray(np.concatenate(
            [r_pf, e_pf, ef, w_flat, scale_b, indh_b, ind_pf_half[c]], axis=1))
        in_maps.append(dict(
            crit=crit, misc=misc,
            y_row=np.ascontiguousarray(ysh.reshape(1, JSHARD)),
        ))
    return in_maps


def kernel(risk_pred, y, e, W, **run_kwargs):
    nc = _get_nc()
    in_maps = make_in_maps(
        np.asarray(risk_pred, np.float32),
        np.asarray(y, np.float32),
        np.asarray(e, np.int32),
        np.asarray(W, np.float32),
    )
    result = run_bass_kernel_spmd(nc, in_maps, core_ids=list(range(NCORES)),
                                  **run_kwargs)
    total = np.float32(0.0)
    for r in result.results:
        total = np.float32(total + r["out"][0, 0])
    kernel.last_result = result
    return np.asarray(total, np.float32)

